# revision 33
# baseline (speedup 1.0000x reference)
"""BitNet MLP (act_quant -> ternary matmul -> relu^2 -> SubLN -> act_quant ->
ternary matmul) on 8 Trainium2 NeuronCores, data-parallel over tokens.

Math notes (exactness):
- act_quant int levels (|q| <= 127) and ternary weights {-1,0,1} are exactly
  representable in bf16, so both matmuls run on the PE in bf16 with exact
  integer arithmetic (f32 PSUM accumulation, |sums| < 2^24).
- All quantization scales are folded into per-token scalars applied to the
  final [tok, 512] output: out = i2 * beta_t with
    beta_t = clip(c_t * alpha_t * Sabs_t, 1e-5) * clip(mean|w_dn|,1e-5) / 127
  where alpha_t = (clip(max|x_t|,1e-5) * clip(mean|w_up|,1e-5) / 127)^2,
  Sabs_t = max_i |relu(ih)^2 * g|, c_t = rsqrt(var_t + 1e-6).
- Rounding uses the magic-number trick (x + 1.5*2^23 - 1.5*2^23) == RNE
  round-to-integer for |x| < 2^22, matching jnp.round (half-to-even).
  HW-verified: both adds can live in ONE tensor_scalar (op0=add M,
  op1=subtract M) because the f32 ALU rounds the intermediate sum.
- SubLN variance is recovered from the quantized intermediate:
  var = alpha^2 * sum(iu^2) * (Sabs/127)^2 / (2048 * g0^2).

Performance structure (measured ~620us vs 984us baseline, 8 cores):
- Software-pipelined emission with explicit skew so the PE's in-order queue
  always has matmul work: per step i the PE receives
  [up_{i+2}] [iuT_i transposes + down_i] [xT_{i+4}].
- Engine balance (per 128-token tile, all three ~96-98% busy):
  PE  ~8.2us: 16 up MM + 16 down MM (N=512) + 20 transposes
  ACT ~8.0us: xq, 2 relu drains, q2=Square(iu)+accum, o drain, xT copy,
              o*beta scale, iuT copies g0/g1
  DVE ~8.2us: x absmax+scale, ix, rmax, (r*dr)*r fused square-scale,
              fused round -> bf16, iuT copies g2/g3, batched beta chain
  (GPSIMD tensor ops are ~15x slower AND degrade other engines' SBUF
  bandwidth - it only issues DMAs.)
- Weight prep: single pass, one 4MB DMA per matrix (16-SDMA parallel) on
  separate queues, quantized from SBUF; the wdn chain runs entirely on DVE
  and the first 3 x-tiles are staged mid-prep from consts buffers so the
  first up-matmul fires at ~51us instead of ~145us.
"""
import os
import numpy as np

import concourse.bass as bass
import concourse.tile as tile
from concourse import mybir
from concourse.bass_utils import run_bass_kernel_spmd
from concourse.masks import make_identity

# ---------------------------------------------------------------------------
# Workaround for walrus "Too many sync wait commands" on the TileContext tail
# drain: split the drain's semaphore waits across single-wait SP NOPs, then
# advance the observed clocks so the real drain needs none.
import re as _re
import bass_rust as _bass_rust


def _patched_drain_and_barrier(self, tick_clock, wait_clock):
    gc = tick_clock.global_clock
    ticks = list(map(int, _re.findall(r"\d+", repr(gc))))
    n = len(ticks)
    nonzero = [(i, t) for i, t in enumerate(ticks) if t > 0]
    for i, t in nonzero:
        sub = [0] * n
        sub[i] = t
        sub_scoped = _bass_rust.ScopedClock({None: _bass_rust.VectorClock(sub)})
        nop = self.nc.sync.nop()
        wait_clock.add_sem_waits(nop.ins, sub_scoped)
        for ec in wait_clock.engine_clocks:
            ec.update_past(sub_scoped)
    drain_inst = self.nc.sync.drain()
    wait_clock.add_sem_waits(drain_inst.ins,
                             _bass_rust.ScopedClock({None: gc}))
    self.nc.all_engine_barrier()
    popped = self.nc._tile_sem_poison_stack.pop()
    assert popped is self._sem_poison
    self.nc.clear_and_free_semaphores(list(self.sems.allocated().values()))
    self.nc.all_engine_barrier()


tile.TileContext._drain_and_barrier = _patched_drain_and_barrier


def _split_sync_waits(nc, keep_default=1):
    """walrus caps the number of semaphore waits a single instruction can
    carry (CTRL ops take only 1; compute ops a few). Hoist excess waits onto
    single-wait NOPs inserted immediately before the instruction on the same
    engine — identical semantics, engines execute in order."""
    import dataclasses
    keep_by_op = {}
    proto = None
    for f in nc.m.functions:
        for bb in f.blocks:
            for inst in bb.instructions:
                if type(inst).__name__ == "InstNoOp":
                    proto = inst
                    break
            if proto is not None:
                break
        if proto is not None:
            break
    counter = [0]
    for f in nc.m.functions:
        new_blocks = []
        for bb in f.blocks:
            out = []
            changed = False
            for inst in bb.instructions:
                si = inst.sync_info
                ow = list(si.on_wait) if si is not None and si.on_wait else []
                keep = keep_by_op.get(inst.opcode, keep_default)
                if len(ow) > keep:
                    assert proto is not None, "no NoOp prototype found yet"
                    for w in ow[:-keep]:
                        counter[0] += 1
                        nop = dataclasses.replace(
                            proto,
                            name=f"I-waitsplit-{counter[0]}",
                            engine=inst.engine,
                            sync_info=_bass_rust.SyncInfo(on_wait=[w],
                                                          on_update=[]),
                        )
                        out.append(nop)
                    si.on_wait = ow[-keep:]
                    changed = True
                out.append(inst)
            if changed:
                bb2 = _bass_rust.BasicBlock(name=bb.name, instructions=out)
                bb2.IsExit = bb.IsExit
                bb2.IsLoopEntry = bb.IsLoopEntry
                bb2.IsPredicated = bb.IsPredicated
                new_blocks.append(bb2)
            else:
                new_blocks.append(bb)
        f.blocks = new_blocks
# ---------------------------------------------------------------------------

F32 = mybir.dt.float32
BF16 = mybir.dt.bfloat16
ALU = mybir.AluOpType
AF = mybir.ActivationFunctionType

N_CORES = 8
B, S, H, I = 8, 8192, 512, 2048
TOK = B * S                  # 65536 tokens total
TPC = TOK // N_CORES         # 8192 tokens per core
P = 128                      # partition tile
NT = TPC // P                # 64 token tiles per core
NKH = H // P                 # 4 k-tiles over H
NKI = I // P                 # 16 k-tiles over I
NB = I // 512                # 4 rhs chunks for the up matmul

MAGIC = 12582912.0           # 1.5 * 2^23: RNE round-to-int trick
EPS = 1e-6                   # SubLN eps (from reference)

LAST_RESULT = None           # set by kernel() for test harness introspection


def _stage_weight(nc, tc, ctx, wT_dram, n_ktiles, fd, name, dma_engine,
                  n_dma=1):
    """Pull the full [n_ktiles*128, fd] matrix into a [128, n_ktiles, fd]
    SBUF tile. n_dma>1 splits the transfer into k-range chunks on the same
    queue so downstream per-chunk consumers start before the tail arrives
    (each InstDMACopy is already 16-SDMA parallel internally).
    """
    stage = ctx.enter_context(tc.tile_pool(name=f"{name}_stage", bufs=1))
    wf = stage.tile([P, n_ktiles, fd], F32, tag="wf", name=f"{name}_wf")
    kq = n_ktiles // n_dma
    for d in range(n_dma):
        k0 = d * kq
        # access pattern: partition p <- row (k*128+p) of wT
        src = bass.AP(tensor=wT_dram[:].tensor, offset=k0 * P * fd,
                      ap=[[fd, P], [P * fd, kq], [1, fd]])
        dma_engine.dma_start(out=wf[:, k0:k0 + kq, :], in_=src)
    return stage, wf


def _quant_weight(nc, tc, ctx, consts, stage, wf, n_ktiles, fd, name,
                  magicb, ps_pool):
    """abs-mean from SBUF (chunked ACT), then round+clip to ternary bf16
    (ACT rt + DVE cl/wq, chunk-pipelined). Critical-path variant (wup)."""
    n_elem = n_ktiles * 128 * fd
    kc = max(1, n_ktiles // 4)
    nchunk = n_ktiles // kc
    asum = consts.tile([P, nchunk], F32, tag=f"{name}_asum")
    jpool = ctx.enter_context(tc.tile_pool(name=f"{name}_j", bufs=2))
    for ci, c in enumerate(range(0, n_ktiles, kc)):
        srcv = wf[:, c:c + kc, :].rearrange("p a b -> p (a b)")
        junk = jpool.tile([P, kc * fd], BF16, tag="junk",
                          name=f"{name}_junk{ci}")
        nc.scalar.activation(out=junk, in_=srcv, func=AF.Abs,
                             accum_out=asum[:, ci:ci + 1])
    tot = consts.tile([P, 1], F32, tag=f"{name}_tot")
    nc.vector.tensor_reduce(out=tot, in_=asum, axis=mybir.AxisListType.X,
                            op=ALU.add)
    swq, meanclip = _wq_scale(nc, tc, ctx, consts, tot, n_elem, name,
                              ps_pool)

    rcpool = ctx.enter_context(tc.tile_pool(name=f"{name}_rc", bufs=2))
    wq = consts.tile([P, n_ktiles, fd], BF16, tag=f"{name}_wq")
    for c in range(0, n_ktiles, kc):
        cfd = kc * fd
        srcv = wf[:, c:c + kc, :].rearrange("p a b -> p (a b)")
        dst = wq[:, c:c + kc, :].rearrange("p a b -> p (a b)")
        rt = rcpool.tile([P, cfd], F32, tag="rt", name=f"{name}_rt{c}")
        nc.scalar.activation(out=rt, in_=srcv, func=AF.Identity,
                             bias=magicb, scale=swq)
        cl = rcpool.tile([P, cfd], F32, tag="cl", name=f"{name}_cl{c}")
        nc.vector.tensor_scalar(out=cl, in0=rt, scalar1=MAGIC, scalar2=1.0,
                                op0=ALU.subtract, op1=ALU.min)
        nc.vector.tensor_scalar(out=dst, in0=cl, scalar1=-1.0,
                                scalar2=None, op0=ALU.max)
    return wq, meanclip


def _quant_weight_dve(nc, tc, ctx, consts, stage, wf, n_ktiles, fd, name,
                      ps_pool):
    """All-DVE variant (wdn): keeps the ACT queue free for the first
    x-tiles. abs via DVE reduce, round via fused (mult,+M) / (-M,min) ts."""
    n_elem = n_ktiles * 128 * fd
    asum = consts.tile([P, 2], F32, tag=f"{name}_asum2")
    kh = n_ktiles // 2
    for h in range(2):
        half = wf[:, h * kh:(h + 1) * kh, :].rearrange("p a b -> p (a b)")
        nc.vector.tensor_reduce(out=asum[:, h:h + 1], in_=half,
                                axis=mybir.AxisListType.X, op=ALU.add,
                                apply_absolute_value=True)
    tot = consts.tile([P, 1], F32, tag=f"{name}_tot")
    nc.vector.tensor_reduce(out=tot, in_=asum, axis=mybir.AxisListType.X,
                            op=ALU.add)
    swq, meanclip = _wq_scale(nc, tc, ctx, consts, tot, n_elem, name,
                              ps_pool)

    rcpool = ctx.enter_context(tc.tile_pool(name=f"{name}_rc", bufs=1))
    wq = consts.tile([P, n_ktiles, fd], BF16, tag=f"{name}_wq")
    kc = max(1, n_ktiles // 4)
    for c in range(0, n_ktiles, kc):
        cfd = kc * fd
        srcv = wf[:, c:c + kc, :].rearrange("p a b -> p (a b)")
        dst = wq[:, c:c + kc, :].rearrange("p a b -> p (a b)")
        rt = rcpool.tile([P, cfd], F32, tag="rt", name=f"{name}_rt{c}")
        nc.vector.tensor_scalar(out=rt, in0=srcv, scalar1=swq,
                                scalar2=MAGIC, op0=ALU.mult, op1=ALU.add)
        cl = rcpool.tile([P, cfd], F32, tag="cl", name=f"{name}_cl{c}")
        nc.vector.tensor_scalar(out=cl, in0=rt, scalar1=MAGIC, scalar2=1.0,
                                op0=ALU.subtract, op1=ALU.min)
        nc.vector.tensor_scalar(out=dst, in0=cl, scalar1=-1.0,
                                scalar2=None, op0=ALU.max)
    return wq, meanclip


def _wq_scale(nc, tc, ctx, consts, tot, n_elem, name, ps_pool):
    """Global abs-mean -> clip -> reciprocal, via ones-matmul broadcast."""
    opool = ctx.enter_context(tc.tile_pool(name=f"{name}_on", bufs=1))
    ones128 = opool.tile([P, P], F32, tag="ones128", name=f"{name}_ones")
    nc.vector.memset(ones128, 1.0)
    totp = ps_pool.tile([P, 1], F32, tag="o", name=f"{name}_totp")
    nc.tensor.matmul(out=totp, lhsT=ones128, rhs=tot, start=True,
                     stop=True)
    gsum = consts.tile([P, 1], F32, tag=f"{name}_gsum")
    nc.scalar.copy(out=gsum, in_=totp)
    meanclip = consts.tile([P, 1], F32, tag=f"{name}_meanclip")
    nc.vector.tensor_scalar(out=meanclip, in0=gsum, scalar1=1.0 / n_elem,
                            scalar2=1e-5, op0=ALU.mult, op1=ALU.max)
    swq = consts.tile([P, 1], F32, tag=f"{name}_swq")
    nc.vector.reciprocal(out=swq, in_=meanclip)
    return swq, meanclip


def build_nc(general_g: bool):
    nc = bass.Bass()
    x_d = nc.dram_tensor("x", [TPC, H], F32, kind="ExternalInput")
    wupT_d = nc.dram_tensor("wupT", [H, I], F32, kind="ExternalInput")
    wdnT_d = nc.dram_tensor("wdnT", [I, H], F32, kind="ExternalInput")
    g_d = nc.dram_tensor("g", [I], F32, kind="ExternalInput")
    out_d = nc.dram_tensor("out", [TPC, H], F32, kind="ExternalOutput")

    from contextlib import ExitStack
    with ExitStack() as ctx:
        tc = ctx.enter_context(tile.TileContext(nc))

        # ---------------- constants / early-tile buffers ----------------
        consts = ctx.enter_context(tc.tile_pool(name="consts", bufs=1))

        ident = consts.tile([P, P], BF16)
        make_identity(nc, ident)

        magicb = consts.tile([P, 1], F32)
        nc.vector.memset(magicb, MAGIC)

        g_bc = None
        if general_g:
            g_bc = consts.tile([P, I], F32)
            g_ap = g_d[:]
            g_bcast_ap = bass.AP(tensor=g_ap.tensor, offset=g_ap.offset,
                                 ap=[[0, P]] + list(g_ap.ap))
            nc.gpsimd.dma_start(out=g_bc, in_=g_bcast_ap)

        BG = 8  # tiles per beta batch
        KV = 1.0 / (127.0 * 127.0 * I)
        NEARLY_A, NEARLY_T = 3, 2   # tiles staged during weight prep

        # PSUM pools (created before prep so early transposes can run)
        ps_ih = ctx.enter_context(tc.tile_pool(name="ps_ih", bufs=2,
                                               space="PSUM"))
        ps_tr = ctx.enter_context(tc.tile_pool(name="ps_tr", bufs=3,
                                               space="PSUM"))
        ps_o = ctx.enter_context(tc.tile_pool(name="ps_o", bufs=1,
                                              space="PSUM"))

        # dedicated consts-resident buffers for tiles staged during prep
        early_a = []
        for j in range(NEARLY_A):
            early_a.append({
                "x": consts.tile([P, H], F32, tag=f"ex{j}", name=f"ex{j}"),
                "xq": consts.tile([P, H], F32, tag=f"exq{j}", name=f"exq{j}"),
                "ix": consts.tile([P, H], BF16, tag=f"eix{j}",
                                  name=f"eix{j}"),
                "xm": consts.tile([P, 1], F32, tag=f"exm{j}", name=f"exm{j}"),
                "xr": consts.tile([P, 1], F32, tag=f"exr{j}", name=f"exr{j}"),
                "xsc": consts.tile([P, 1], F32, tag=f"exs{j}",
                                   name=f"exs{j}"),
            })
        early_t = [consts.tile([P, NKH, P], BF16, tag=f"exT{j}",
                               name=f"exT{j}") for j in range(NEARLY_T)]

        # per-tile state handles + batch-0 stat tiles (used during prep)
        st = {}
        st[("bt", 0)] = (
            consts.tile([P, BG], F32, tag="t08_e", name="t08_e"),
            consts.tile([P, BG], F32, tag="Sm28_e", name="Sm28_e"),
            consts.tile([P, BG], F32, tag="q28_e", name="q28_e"),
        )

        def batch_slots(j):
            b = j // BG
            if ("bt", b) not in st:
                st[("bt", b)] = (
                    statp.tile([P, BG], F32, tag="t08", name=f"t08_{b}"),
                    statp.tile([P, BG], F32, tag="Sm28", name=f"Sm28_{b}"),
                    statp.tile([P, BG], F32, tag="q28", name=f"q28_{b}"),
                )
            return st[("bt", b)]

        def emit_A_core(j, x_sb, xq, ix, xm, xr, xsc, ix_on_gps):
            t08, _, _ = batch_slots(j)
            sl = j % BG
            nc.sync.dma_start(out=x_sb, in_=x_d[j * P:(j + 1) * P, :])
            nc.vector.tensor_reduce(out=xm, in_=x_sb,
                                    axis=mybir.AxisListType.X, op=ALU.max,
                                    apply_absolute_value=True)
            # slot holds clip(max|x|,1e-5)/127 so xsc is a single reciprocal
            # (beta compensates by using meanclip in place of meanclip/127)
            nc.vector.tensor_scalar(out=t08[:, sl:sl + 1], in0=xm,
                                    scalar1=1.0 / 127.0, scalar2=1e-5 / 127.0,
                                    op0=ALU.mult, op1=ALU.max)
            nc.vector.reciprocal(out=xsc, in_=t08[:, sl:sl + 1])
            nc.scalar.activation(out=xq, in_=x_sb, func=AF.Identity,
                                 bias=magicb, scale=xsc)
            if ix_on_gps:
                nc.gpsimd.tensor_scalar(out=ix, in0=xq, scalar1=MAGIC,
                                        scalar2=None, op0=ALU.subtract)
            else:
                nc.vector.tensor_scalar(out=ix, in0=xq, scalar1=MAGIC,
                                        scalar2=None, op0=ALU.subtract)
            st[("ix", j)] = ix

        def emit_T_core(j, xT_sb):
            ix = st.pop(("ix", j))
            xT_ps = ps_tr.tile([P, 8, P], BF16, tag="tr")
            for k in range(NKH):
                nc.tensor.transpose(out=xT_ps[:, k, :],
                                    in_=ix[:, k * P:(k + 1) * P],
                                    identity=ident)
            nc.scalar.copy(out=xT_sb, in_=xT_ps[:, 0:NKH, :])
            st[("xT", j)] = xT_sb

        # ---------------- weight prep (x tiles 0-2 interleaved) ----------
        g0b = consts.tile([P, 1], F32)
        with ExitStack() as pctx:
            gp = pctx.enter_context(tc.tile_pool(name="gprep", bufs=1))
            ones_row = gp.tile([1, P], F32, tag="ones_row")
            nc.vector.memset(ones_row, 1.0)
            g0_sb = gp.tile([1, 1], F32, tag="g0sb")
            nc.gpsimd.dma_start(out=g0_sb, in_=g_d[0:1])
            g0_ps = ps_o.tile([P, 1], F32, tag="o", name="g0_ps")
            nc.tensor.matmul(out=g0_ps, lhsT=ones_row, rhs=g0_sb,
                             start=True, stop=True)
            nc.scalar.copy(out=g0b, in_=g0_ps)

            # both weight DMAs issued up front on separate queues
            up_stage, up_wf = _stage_weight(nc, tc, pctx, wupT_d, NKH,
                                            NB * 512, "wup", nc.gpsimd,
                                            n_dma=2)
            dn_stage, dn_wf = _stage_weight(nc, tc, pctx, wdnT_d, NKI,
                                            512, "wdn", nc.scalar, n_dma=2)
            # wup on the ACT-fast path (gates the first up matmul)
            wup_q, up_meanclip = _quant_weight(
                nc, tc, pctx, consts, up_stage, up_wf, NKH, NB * 512,
                "wup", magicb, ps_o)
            # first x tiles: their ACT/DVE ops queue right after wup quant,
            # ahead of the all-DVE wdn quant chain
            for j in range(NEARLY_A):
                e = early_a[j]
                emit_A_core(j, e["x"], e["xq"], e["ix"], e["xm"], None,
                            e["xsc"], ix_on_gps=False)
            for j in range(NEARLY_T):
                emit_T_core(j, early_t[j])
            # wdn quant entirely on DVE (ACT stays free for rdrains/xq)
            wdn_q, dn_meanclip = _quant_weight_dve(
                nc, tc, pctx, consts, dn_stage, dn_wf, NKI, 512,
                "wdn", ps_o)

            # k1b = clip(mean|w_up|,1e-5)/127  (per-token gamma multiplier)
            k1b = consts.tile([P, 1], F32)
            nc.vector.tensor_scalar_mul(out=k1b, in0=up_meanclip,
                                        scalar1=1.0 / 127.0)
            # wdk = clip(mean|w_dn|,1e-5)/127  (final output multiplier)
            wdk = consts.tile([P, 1], F32)
            nc.vector.tensor_scalar_mul(out=wdk, in0=dn_meanclip,
                                        scalar1=1.0 / 127.0)
            # sg127 = sign(g0)*127, g0a = |g0|
            sg127 = consts.tile([P, 1], F32)
            nc.scalar.activation(out=sg127, in_=g0b, func=AF.Sign)
            nc.vector.tensor_scalar_mul(out=sg127, in0=sg127, scalar1=127.0)
            g0a = consts.tile([P, 1], F32)
            nc.scalar.activation(out=g0a, in_=g0b, func=AF.Abs)

            # isg = sign(g0)/127 (or 1/127 for general g)
            isg = consts.tile([P, 1], F32)
            if general_g:
                nc.vector.memset(isg, 1.0 / 127.0)
            else:
                nc.vector.tensor_scalar_mul(out=isg, in0=sg127,
                                            scalar1=1.0 / (127.0 * 127.0))

        # ---------------- pools ----------------
        xs_pool = ctx.enter_context(tc.tile_pool(name="xs", bufs=4))
        xq_pool = ctx.enter_context(tc.tile_pool(name="xqp", bufs=2))
        ix_pool = ctx.enter_context(tc.tile_pool(name="ixp", bufs=4))
        xT_pool = ctx.enter_context(tc.tile_pool(name="xTp", bufs=5))
        r_pool = ctx.enter_context(tc.tile_pool(name="rp", bufs=3))
        rtp_pool = ctx.enter_context(tc.tile_pool(name="rtpp", bufs=1))
        rt_pool = None
        if general_g:
            rt_pool = ctx.enter_context(tc.tile_pool(name="rtp", bufs=2))
        iu_pool = ctx.enter_context(tc.tile_pool(name="iup", bufs=3))
        iuT_pool = ctx.enter_context(tc.tile_pool(name="iuTp", bufs=2))
        o_pool = ctx.enter_context(tc.tile_pool(name="op", bufs=11))
        o2_pool = ctx.enter_context(tc.tile_pool(name="o2p", bufs=4))
        small = ctx.enter_context(tc.tile_pool(name="small", bufs=8))
        statp = ctx.enter_context(tc.tile_pool(name="statp", bufs=8))
        betap = ctx.enter_context(tc.tile_pool(name="betap", bufs=3))
        junkp = ctx.enter_context(tc.tile_pool(name="mjunk", bufs=1))

        # junk elementwise target for the q2 ACT square-accumulate
        junkq2 = None
        if not general_g:
            junkq2 = junkp.tile([P, I], BF16, tag="junkq2")

        def emit_A(j):
            """x DMA + absmax + per-token scale + quantize to int levels."""
            x_sb = xs_pool.tile([P, H], F32, tag="x")
            xq = xq_pool.tile([P, H], F32, tag="xq")
            ix = ix_pool.tile([P, H], BF16, tag="ix")
            xm = small.tile([P, 1], F32, tag="xm")
            xsc = small.tile([P, 1], F32, tag="xsc")
            emit_A_core(j, x_sb, xq, ix, xm, None, xsc, ix_on_gps=False)

        def emit_T(j):
            """PE transpose of ix (end of PE stream) + ACT copy to SBUF."""
            xT_sb = xT_pool.tile([P, NKH, P], BF16, tag="xTsb")
            emit_T_core(j, xT_sb)

        def emit_up(j):
            """Up matmul (2 halves of 2 banks each)."""
            xT_sb = st.pop(("xT", j))
            r_sb = r_pool.tile([P, I], F32, tag="r")
            for h in range(2):
                ihh = ps_ih.tile([P, 1024], F32, tag="ih")
                for nb in range(2):
                    lo = nb * 512
                    for k in range(NKH):
                        nc.tensor.matmul(
                            out=ihh[:, lo:lo + 512],
                            lhsT=xT_sb[:, k, :],
                            rhs=wup_q[:, k, (2 * h + nb) * 512:
                                      (2 * h + nb + 1) * 512],
                            start=(k == 0), stop=(k == NKH - 1))
                st[("ih", j, h)] = ihh
            st[("r", j)] = r_sb

        def emit_rdrain(j, h):
            r_sb = st[("r", j)]
            ihh = st.pop(("ih", j, h))
            nc.scalar.activation(out=r_sb[:, h * 1024:(h + 1) * 1024],
                                 in_=ihh, func=AF.Relu)

        def emit_C(j):
            """Elementwise chain: quant scale from max(r), fused round, q2."""
            _, Sm28, q28 = batch_slots(j)
            sl = j % BG
            r_sb = st.pop(("r", j))
            if general_g:
                s = rtp_pool.tile([P, I], F32, tag="s")
                nc.vector.scalar_tensor_tensor(out=s, in0=r_sb, scalar=1.0,
                                               in1=r_sb, op0=ALU.mult,
                                               op1=ALU.mult)
                sq_in = rt_pool.tile([P, I], F32, tag="sqin")
                nc.vector.tensor_tensor(out=sq_in, in0=s, in1=g_bc,
                                        op=ALU.mult)
                nc.vector.tensor_reduce(out=Sm28[:, sl:sl + 1], in_=sq_in,
                                        axis=mybir.AxisListType.X,
                                        op=ALU.max,
                                        apply_absolute_value=True)
                junk3 = junkp.tile([P, I], BF16, tag="junk3")
                nc.scalar.activation(out=junk3, in_=s, func=AF.Square,
                                     accum_out=q28[:, sl:sl + 1])
                sc2 = small.tile([P, 1], F32, tag="sc2")
                nc.vector.tensor_scalar(out=sc2, in0=Sm28[:, sl:sl + 1],
                                        scalar1=1e-30, scalar2=isg,
                                        op0=ALU.max, op1=ALU.mult)
                dr = small.tile([P, 1], F32, tag="dr")
                nc.vector.reciprocal(out=dr, in_=sc2)
                rt = rt_pool.tile([P, I], F32, tag="rt")
                nc.vector.tensor_scalar(out=rt, in0=sq_in, scalar1=dr,
                                        scalar2=MAGIC, op0=ALU.mult,
                                        op1=ALU.add)
                iu = iu_pool.tile([P, I], BF16, tag="iu")
                nc.vector.tensor_scalar(out=iu, in0=rt, scalar1=MAGIC,
                                        scalar2=None, op0=ALU.subtract)
            else:
                # Sabs-equivalent: max(relu(ih)) (>= 0); d folded via isg
                nc.vector.tensor_reduce(out=Sm28[:, sl:sl + 1], in_=r_sb,
                                        axis=mybir.AxisListType.X, op=ALU.max)
                mr = small.tile([P, 1], F32, tag="mr")
                nc.vector.tensor_scalar_max(out=mr, in0=Sm28[:, sl:sl + 1],
                                            scalar1=1e-15)
                sc2 = small.tile([P, 1], F32, tag="sc2")
                nc.vector.tensor_scalar(out=sc2, in0=mr, scalar1=mr,
                                        scalar2=isg, op0=ALU.mult,
                                        op1=ALU.mult)
                dr = small.tile([P, 1], F32, tag="dr")
                nc.vector.reciprocal(out=dr, in_=sc2)
                # ds = (r*dr)*r = d*relu(ih)^2 in one fused DVE op
                rtp = rtp_pool.tile([P, I], F32, tag="rtp")
                nc.vector.scalar_tensor_tensor(out=rtp, in0=r_sb, scalar=dr,
                                               in1=r_sb, op0=ALU.mult,
                                               op1=ALU.mult)
                # fused RNE round: (+M, -M) in one op (f32 ALU rounds the
                # intermediate), output straight to bf16 int levels
                iu = iu_pool.tile([P, I], BF16, tag="iu")
                nc.vector.tensor_scalar(out=iu, in0=rtp, scalar1=MAGIC,
                                        scalar2=MAGIC, op0=ALU.add,
                                        op1=ALU.subtract)
                # q2 = sum(iu^2) on ACT (Square + accumulator)
                nc.scalar.activation(out=junkq2, in_=iu, func=AF.Square,
                                     accum_out=q28[:, sl:sl + 1])
            st[("iu", j)] = iu

        def emit_D(j):
            """iu transposes (2 PSUM tiles of 8) + split copies + down mm."""
            iu = st.pop(("iu", j))
            iuT_sb = iuT_pool.tile([P, NKI, P], BF16, tag="iuTsb")
            trA = ps_tr.tile([P, 8, P], BF16, tag="tr")
            trB = ps_tr.tile([P, 8, P], BF16, tag="tr")
            for k in range(NKI):
                trp = trA if k < 8 else trB
                nc.tensor.transpose(out=trp[:, k % 8, :],
                                    in_=iu[:, k * P:(k + 1) * P],
                                    identity=ident)
            # copies in 4 groups of 4 k-tiles: g0,g1 on ACT; g2,g3 on DVE
            nc.scalar.copy(out=iuT_sb[:, 0:4, :], in_=trA[:, 0:4, :])
            nc.scalar.copy(out=iuT_sb[:, 4:8, :], in_=trA[:, 4:8, :])
            nc.vector.tensor_copy(out=iuT_sb[:, 8:12, :], in_=trB[:, 0:4, :])
            nc.vector.tensor_copy(out=iuT_sb[:, 12:16, :], in_=trB[:, 4:8, :])
            o_ps = ps_o.tile([P, H], F32, tag="o")
            for k in range(NKI):
                nc.tensor.matmul(out=o_ps, lhsT=iuT_sb[:, k, :],
                                 rhs=wdn_q[:, k, :],
                                 start=(k == 0), stop=(k == NKI - 1))
            st[("ops", j)] = o_ps

        def emit_odrain(j):
            o_ps = st.pop(("ops", j))
            o_sb = o_pool.tile([P, H], F32, tag="osb")
            nc.scalar.copy(out=o_sb, in_=o_ps)
            st[("o", j)] = o_sb

        def emit_beta(jhi):
            """Batched beta chain for tiles jhi-7..jhi -> per-batch b8."""
            t08, Sm28, q28 = st.pop(("bt", jhi // BG))
            scc8 = betap.tile([P, BG], F32, tag="scc8")
            if general_g:
                nc.vector.tensor_scalar_max(out=scc8, in0=Sm28, scalar1=1e-30)
            else:
                # Sm28 holds max(relu(ih)) >= 0; Sabs = Sm^2
                ssq8 = betap.tile([P, BG], F32, tag="ssq8")
                nc.vector.tensor_tensor(out=ssq8, in0=Sm28, in1=Sm28,
                                        op=ALU.mult)
                nc.vector.tensor_scalar_max(out=scc8, in0=ssq8, scalar1=1e-30)
            ga8 = betap.tile([P, BG], F32, tag="ga8")
            nc.vector.tensor_scalar_mul(out=ga8, in0=t08,
                                        scalar1=up_meanclip)
            al8 = betap.tile([P, BG], F32, tag="al8")
            nc.vector.tensor_tensor(out=al8, in0=ga8, in1=ga8, op=ALU.mult)
            m18 = betap.tile([P, BG], F32, tag="m18")
            nc.vector.tensor_tensor(out=m18, in0=al8, in1=scc8, op=ALU.mult)
            v18 = betap.tile([P, BG], F32, tag="v18")
            Ve8 = betap.tile([P, BG], F32, tag="Ve8")
            if general_g:
                al28 = betap.tile([P, BG], F32, tag="al28")
                nc.vector.tensor_tensor(out=al28, in0=al8, in1=al8,
                                        op=ALU.mult)
                nc.vector.tensor_tensor(out=v18, in0=al28, in1=q28,
                                        op=ALU.mult)
                nc.vector.tensor_scalar(out=Ve8, in0=v18, scalar1=1.0 / I,
                                        scalar2=EPS, op0=ALU.mult,
                                        op1=ALU.add)
            else:
                m28 = betap.tile([P, BG], F32, tag="m28")
                nc.vector.tensor_tensor(out=m28, in0=m18, in1=m18,
                                        op=ALU.mult)
                nc.vector.tensor_tensor(out=v18, in0=m28, in1=q28,
                                        op=ALU.mult)
                nc.vector.tensor_scalar(out=Ve8, in0=v18, scalar1=KV,
                                        scalar2=EPS, op0=ALU.mult,
                                        op1=ALU.add)
            sq8 = betap.tile([P, BG], F32, tag="sq8")
            nc.scalar.activation(out=sq8, in_=Ve8, func=AF.Sqrt)
            cr8 = betap.tile([P, BG], F32, tag="cr8")
            nc.vector.reciprocal(out=cr8, in_=sq8)
            h18 = betap.tile([P, BG], F32, tag="h18")
            nc.vector.tensor_tensor(out=h18, in0=cr8, in1=cr8, op=ALU.mult)
            h28 = betap.tile([P, BG], F32, tag="h28")
            nc.vector.tensor_tensor(out=h28, in0=h18, in1=Ve8, op=ALU.mult)
            h38 = betap.tile([P, BG], F32, tag="h38")
            nc.vector.tensor_scalar(out=h38, in0=h28, scalar1=-0.5,
                                    scalar2=1.5, op0=ALU.mult, op1=ALU.add)
            c8 = betap.tile([P, BG], F32, tag="c8")
            nc.vector.tensor_tensor(out=c8, in0=cr8, in1=h38, op=ALU.mult)
            if general_g:
                m1g8 = m18
            else:
                m1g8 = betap.tile([P, BG], F32, tag="m1g8")
                nc.vector.tensor_scalar_mul(out=m1g8, in0=m18, scalar1=g0a)
            mu8 = betap.tile([P, BG], F32, tag="mu8")
            nc.vector.tensor_tensor(out=mu8, in0=c8, in1=m1g8, op=ALU.mult)
            b8 = statp.tile([P, BG], F32, tag="b8",
                            name=f"b8_{jhi // BG}")
            nc.vector.tensor_scalar(out=b8, in0=mu8, scalar1=1e-5,
                                    scalar2=wdk, op0=ALU.max, op1=ALU.mult)
            st[("b8", jhi // BG)] = b8

        def emit_o2(j):
            """Scale tile j's output by beta on ACT and store (one per step
            to keep the load flat)."""
            b8 = st[("b8", j // BG)]
            o_sb = st.pop(("o", j))
            o2 = o2_pool.tile([P, H], F32, tag="o2")
            nc.scalar.activation(out=o2, in_=o_sb, func=AF.Identity,
                                 scale=b8[:, j % BG:j % BG + 1])
            nc.sync.dma_start(out=out_d[j * P:(j + 1) * P, :], in_=o2)
            if j % BG == BG - 1:
                del st[("b8", j // BG)]

        # ---------------- software-pipelined emission ----------------
        # per step i the PE queue receives: up_{i+2} | iuT_i+down_i | xT_{i+4}
        # A(0..2) and T(0..1) were emitted during weight prep.
        o2q = []
        for i in range(-2, NT + BG):
            if 3 <= i + 5 < NT:
                emit_A(i + 5)
            if 0 <= i + 2 < NT:
                emit_up(i + 2)
                emit_rdrain(i + 2, 0)
            if 0 <= i < NT:
                emit_D(i)
            if 0 <= i + 2 < NT:
                emit_rdrain(i + 2, 1)
            if 0 <= i + 1 < NT:
                emit_C(i + 1)
            if 2 <= i + 4 < NT:
                emit_T(i + 4)
            if 0 <= i < NT:
                emit_odrain(i)
            if 0 <= i < NT and i % BG == BG - 1:
                emit_beta(i)
            if 0 <= i - BG < NT:
                o2q.append(i - BG)
            for _ in range(1 if i < NT else 4):
                if o2q:
                    emit_o2(o2q.pop(0))

    _split_sync_waits(nc)
    return nc


_NC_CACHE = {}


def kernel(x, w_up, w_down, g):
    global LAST_RESULT
    x = np.ascontiguousarray(x, dtype=np.float32)
    w_up = np.ascontiguousarray(w_up, dtype=np.float32)
    w_down = np.ascontiguousarray(w_down, dtype=np.float32)
    g = np.ascontiguousarray(g, dtype=np.float32)

    if abs(float(g[0])) < 1e-30 and np.all(g == g[0]):
        return np.zeros_like(x)

    general = not bool(np.all(g == g[0]))
    key = ("gen" if general else "const")
    if key not in _NC_CACHE:
        _NC_CACHE[key] = build_nc(general)
    nc = _NC_CACHE[key]

    xt = x.reshape(TOK, H)
    wupT = np.ascontiguousarray(w_up.T)    # [H, I]
    wdnT = np.ascontiguousarray(w_down.T)  # [I, H]
    in_maps = [
        {"x": xt[c * TPC:(c + 1) * TPC], "wupT": wupT, "wdnT": wdnT, "g": g}
        for c in range(N_CORES)
    ]
    res = run_bass_kernel_spmd(
        nc, in_maps, list(range(N_CORES)),
        trace=bool(os.environ.get("BASS_TRACE")),
    )
    LAST_RESULT = res
    out = np.concatenate([res.results[c]["out"] for c in range(N_CORES)],
                         axis=0)
    return out.reshape(B, S, H)
